# revision 26
# baseline (speedup 1.0000x reference)
"""Bass/Trainium2 kernel for nn_ArtistBERT (dense transformer, 6 layers, D=128, S=17).

Data-parallel across 8 NeuronCores: batch is padded 16384 -> 16576 samples and
split 2128/core. Per core, samples are processed in 76 tiles of 28 samples
(4 groups of 7 samples = 119 tokens each). The residual stream lives in
"natural" token-major layout [119 tokens, 4*128] so LayerNorm / softmax-scale /
mask are free-axis ops on DVE/ACT; matmuls take transposed operands produced by
PE transposes. All matmul operands are bf16 (PSUM accumulates f32).

Attention per group g (119 tokens, block-diagonal over 7 samples):
  scoresT[k,q] = kb_g^T-contraction  (lhsT=kb_g, rhs=qb_g)
  pm = exp(scoresT - 30) * blockmask           (softmax max-sub replaced by
  attn_unnorm[q,d] = pm^T @ v_nat               constant shift; normalization
  rowsum[q] = pm^T @ ones                       deferred and folded with the
  y = attn_unnorm * mask/rowsum + x (residual)  input keep-mask)
"""

import os
import sys

import numpy as np

try:
    import concourse.bass as bass  # noqa: F401
except ImportError:  # harness runs from a bare directory
    sys.path.insert(0, "/opt/trn_rl_repo")

import ml_dtypes
from contextlib import ExitStack

import concourse.bass as bass
import concourse.tile as tile
from concourse import bacc, mybir
from concourse import bass_utils

# Force every ACT instruction onto the natural_log_exp_and_others table set
# (index 6 in act_info.json): it contains every function this kernel uses
# (exp, ln, square, relu, copy, identity), so the table load hoists out of
# the tile loop instead of thrashing ~2.7us per set switch.
_ONESET = "natural_log_exp_and_others"
_orig_gat = bacc.get_activation_tables


def _gat_oneset(arch):
    t = _orig_gat(arch)
    assert _ONESET in t
    return {k: (v if k == _ONESET else set()) for k, v in t.items()}


bacc.get_activation_tables = _gat_oneset

F32 = mybir.dt.float32
F32R = mybir.dt.float32r
BF16 = mybir.dt.bfloat16
NPBF = ml_dtypes.bfloat16
AF = mybir.ActivationFunctionType
ALU = mybir.AluOpType

B, S, V, D, NSEG, L = 16384, 17, 29, 128, 4, 6
EPS = 1e-5
EXPC = 30.0  # constant shift inside exp(); cancels in the softmax ratio

NCORES = 8
G = 7            # samples per attention group
GT = G * S       # 119 tokens per group
NG = 4           # groups per tile
TS = NG * G      # 28 samples per tile
NT = 76          # tiles per core (divisible by the 4-wide interleave)
SPC = NT * TS    # 2072 samples per core
BPAD = NCORES * SPC  # 16576
XW = V + NSEG    # 33 fused X|seg feature width


def build_graph(n_tiles, use_bv, use_bd, use_g1b1, use_g2b2, use_bo):
    nc = bacc.Bacc(
        "TRN2",
        target_bir_lowering=False,
        debug=False,
        enable_asserts=True,
        num_devices=NCORES,
    )
    rows = n_tiles * GT
    GTP = GT  # group stride in transposed layout
    TW = NG * GTP

    xc_d = nc.dram_tensor("xc", [rows, NG * XW], F32, kind="ExternalInput").ap()
    mk_d = nc.dram_tensor("mk", [rows, NG], F32, kind="ExternalInput").ap()
    wqkd_d = nc.dram_tensor("wqkd", [D, L * 3 * D], F32R, kind="ExternalInput").ap()
    wv_d = nc.dram_tensor("wv", [D, L * D], BF16, kind="ExternalInput").ap()
    wcat_d = nc.dram_tensor("wcat", [XW, D], F32R, kind="ExternalInput").ap()
    wo_d = nc.dram_tensor("wo", [D, V], BF16, kind="ExternalInput").ap()
    iden_d = nc.dram_tensor("iden", [GT, GT], F32, kind="ExternalInput").ap()
    bm_d = nc.dram_tensor("bm", [GT, TW], BF16, kind="ExternalInput").ap()
    onec_d = nc.dram_tensor("onec", [GT, 1], BF16, kind="ExternalInput").ap()
    pos_d = nc.dram_tensor("pos4", [GT, NG * D], F32, kind="ExternalInput").ap()
    negc_d = nc.dram_tensor("negc", [GT, 1], F32, kind="ExternalInput").ap()
    bqk_d = nc.dram_tensor("bqk", [D, 2 * L], F32, kind="ExternalInput").ap()
    bv_d = bd_d = g1_d = b1_d = g2_d = b2_d = bo_d = None
    if use_bv:
        bv_d = nc.dram_tensor("bvt", [L * GT, D], F32, kind="ExternalInput").ap()
    if use_bd:
        bd_d = nc.dram_tensor("bdt", [L * GT, D], F32, kind="ExternalInput").ap()
    if use_g1b1:
        g1_d = nc.dram_tensor("g1t", [L * GT, D], F32, kind="ExternalInput").ap()
        b1_d = nc.dram_tensor("b1t", [L * GT, D], F32, kind="ExternalInput").ap()
    if use_g2b2:
        g2_d = nc.dram_tensor("g2t", [L * GT, D], F32, kind="ExternalInput").ap()
        b2_d = nc.dram_tensor("b2t", [L * GT, D], F32, kind="ExternalInput").ap()
    if use_bo:
        bo_d = nc.dram_tensor("bot", [GT, V], F32, kind="ExternalInput").ap()
    out_d = nc.dram_tensor("out", [rows, NG * V], F32, kind="ExternalOutput").ap()

    with tile.TileContext(nc) as tc, ExitStack() as ctx:
        cp = ctx.enter_context(tc.tile_pool(name="consts", bufs=1))
        wp = ctx.enter_context(tc.tile_pool(name="work", bufs=4))
        sp = ctx.enter_context(tc.tile_pool(name="small", bufs=8))
        pp = ctx.enter_context(tc.tile_pool(name="ps", bufs=4, space="PSUM"))
        pn = ctx.enter_context(tc.tile_pool(name="pn", bufs=4, space="PSUM"))

        def cload(name, ap_d, shape, dt):
            t = cp.tile(shape, dt, tag=name)
            nc.sync.dma_start(t[:], ap_d)
            return t

        wqkd = cload("wqkd", wqkd_d, [D, L * 3 * D], F32R)
        wv = cload("wv", wv_d, [D, L * D], BF16)
        wcat = cload("wcat", wcat_d, [XW, D], F32R)
        wo = cload("wo", wo_d, [D, V], BF16)
        iden = cload("iden", iden_d, [GT, GT], F32)
        bm = cload("bm", bm_d, [GT, TW], BF16)
        onec = cload("onec", onec_d, [GT, 1], BF16)
        pos4 = cload("pos4", pos_d, [GT, NG * D], F32)
        negc = cload("negc", negc_d, [GT, 1], F32)
        bqk = cload("bqk", bqk_d, [D, 2 * L], F32)
        bvt = cload("bvt", bv_d, [L * GT, D], F32) if use_bv else None
        bdt = cload("bdt", bd_d, [L * GT, D], F32) if use_bd else None
        g1t = cload("g1t", g1_d, [L * GT, D], F32) if use_g1b1 else None
        b1t = cload("b1t", b1_d, [L * GT, D], F32) if use_g1b1 else None
        g2t = cload("g2t", g2_d, [L * GT, D], F32) if use_g2b2 else None
        b2t = cload("b2t", b2_d, [L * GT, D], F32) if use_g2b2 else None
        bot = cload("bot", bo_d, [GT, V], F32) if use_bo else None

        def bc3(t2d, w):
            # [GT, NG] -> [GT, NG, w] with 0-stride last dim
            return t2d[:].unsqueeze(2).broadcast_to([GT, NG, w])

        def bcg(t2d, w):
            # [GT, w] const -> [GT, NG, w] broadcast over groups
            return t2d.unsqueeze(1).broadcast_to([GT, NG, w])

        def layer_norm(y, gt, bt, out_f32):
            """y: sbuf f32 [GT, NG*D] -> out_f32. var = E[y^2] - mu^2."""
            y3 = y[:].rearrange("p (g d) -> p g d", g=NG)
            s1 = sp.tile([GT, NG], F32, tag="s1")
            nc.vector.reduce_sum(s1[:], y3, axis=mybir.AxisListType.X)
            mu = sp.tile([GT, NG], F32, tag="mu")
            nc.vector.tensor_scalar(mu[:], s1[:], 1.0 / D, None, op0=ALU.mult)
            sq = wp.tile([GT, NG * D], F32, tag="sq")
            nc.scalar.activation(sq[:], y[:], AF.Square)
            s2 = sp.tile([GT, NG], F32, tag="s2")
            nc.vector.reduce_sum(
                s2[:], sq[:].rearrange("p (g d) -> p g d", g=NG),
                axis=mybir.AxisListType.X,
            )
            mu2 = sp.tile([GT, NG], F32, tag="mu2")
            nc.vector.tensor_mul(mu2[:], mu[:], mu[:])
            vv = sp.tile([GT, NG], F32, tag="vv")
            nc.vector.tensor_scalar(
                vv[:], s2[:], 1.0 / D, EPS, op0=ALU.mult, op1=ALU.add
            )
            nc.vector.tensor_sub(vv[:], vv[:], mu2[:])
            # rstd = exp(-0.5*ln(var)): keeps every ACT func in the
            # natural_log_exp table set (no table-switch stalls)
            lnv = sp.tile([GT, NG], F32, tag="lnv")
            nc.scalar.activation(lnv[:], vv[:], AF.Ln)
            rst = sp.tile([GT, NG], F32, tag="rst")
            nc.scalar.activation(rst[:], lnv[:], AF.Exp, scale=-0.5)
            for g in range(NG):
                nc.vector.tensor_scalar(
                    out_f32[:, g * D:(g + 1) * D], y[:, g * D:(g + 1) * D],
                    mu[:, g:g + 1], rst[:, g:g + 1],
                    op0=ALU.subtract, op1=ALU.mult,
                )
            o3 = out_f32[:].rearrange("p (g d) -> p g d", g=NG)
            if gt is not None:
                nc.vector.tensor_tensor(o3, o3, bcg(gt, D), op=ALU.mult)
            if bt is not None:
                nc.vector.tensor_tensor(o3, o3, bcg(bt, D), op=ALU.add)

        def transpose4(src_f32, tag):
            """src [GT, NG*D] f32 -> f32r [D, TW]; group g at cols g*GTP,
            col 119 of each group is a defined zero (iden pad column)."""
            pt = pp.tile([D, TW], F32, tag="ps")
            for g in range(NG):
                nc.tensor.transpose(
                    pt[:, g * GTP:(g + 1) * GTP], src_f32[:, g * D:(g + 1) * D],
                    iden[:],
                )
            xbT = wp.tile([D, TW], F32R, tag=tag + "T")
            nc.scalar.copy(xbT[:], pt[:])
            return xbT

        def dma_embed(it):
            xc = wp.tile([GT, NG * XW], F32, tag="xc")
            nc.sync.dma_start(xc[:], xc_d[bass.ts(it, GT), :])
            mk = wp.tile([GT, NG], F32, tag="mk")
            nc.sync.dma_start(mk[:], mk_d[bass.ts(it, GT), :])

            pxt = pp.tile([XW, TW], F32, tag="ps")
            for g in range(NG):
                nc.tensor.transpose(
                    pxt[:, g * GTP:(g + 1) * GTP], xc[:, g * XW:(g + 1) * XW],
                    iden[:],
                )
            xcT = wp.tile([XW, TW], F32R, tag="xcT")
            nc.scalar.copy(xcT[:], pxt[:])
            px = pn.tile([GT, NG * D], F32, tag="pn")
            for g in range(NG):
                nc.tensor.matmul(
                    px[:, g * D:(g + 1) * D],
                    xcT[:, g * GTP:g * GTP + GT],
                    wcat[:],
                )
            x = wp.tile([GT, NG * D], F32, tag="x")
            nc.vector.tensor_add(x[:], px[:], pos4[:])
            return x, mk

        def layer_fn(l, x, mk):
            wq = wqkd[:, (l * 3 + 0) * D:(l * 3 + 1) * D]
            wk = wqkd[:, (l * 3 + 1) * D:(l * 3 + 2) * D]
            wd = wqkd[:, (l * 3 + 2) * D:(l * 3 + 3) * D]
            wvl = wv[:, l * D:(l + 1) * D]

            xbT = transpose4(x, "x1")

            pq = pp.tile([D, TW], F32, tag="ps")
            nc.tensor.matmul(pq[:], wq, xbT[:])
            qb = wp.tile([D, TW], F32, tag="qb")
            nc.scalar.activation(qb[:], pq[:], AF.Relu, bias=bqk[:, l:l + 1])
            pk = pp.tile([D, TW], F32, tag="ps")
            nc.tensor.matmul(pk[:], wk, xbT[:])
            kb = wp.tile([D, TW], F32, tag="kb")
            nc.scalar.activation(kb[:], pk[:], AF.Relu, bias=bqk[:, L + l:L + l + 1])

            xbTb = wp.tile([D, TW], BF16, tag="xbTb")
            nc.vector.tensor_copy(xbTb[:], xbT[:])
            pv = pn.tile([GT, NG * D], F32, tag="pn")
            for g in range(NG):
                nc.tensor.matmul(
                    pv[:, g * D:(g + 1) * D],
                    xbTb[:, g * GTP:g * GTP + GT], wvl,
                )
            vb = wp.tile([GT, NG * D], BF16, tag="vb")
            if use_bv:
                vf = wp.tile([GT, NG * D], F32, tag="vf")
                v3 = vf[:].rearrange("p (g d) -> p g d", g=NG)
                nc.vector.tensor_tensor(
                    v3, pv[:].rearrange("p (g d) -> p g d", g=NG),
                    bcg(bvt[l * GT:(l + 1) * GT, :], D), op=ALU.add,
                )
                nc.scalar.activation(vb[:], vf[:], AF.Relu)
            else:
                nc.scalar.activation(vb[:], pv[:], AF.Relu)

            pss = pn.tile([GT, TW], F32, tag="pn")
            for g in range(NG):
                nc.tensor.matmul(
                    pss[:, g * GTP:(g + 1) * GTP],
                    kb[:, g * GTP:g * GTP + GT],
                    qb[:, g * GTP:(g + 1) * GTP],
                )
            pme = wp.tile([GT, TW], BF16, tag="pme")
            nc.scalar.activation(pme[:], pss[:], AF.Exp, bias=negc[:])
            pm = wp.tile([GT, TW], BF16, tag="pm")
            nc.vector.tensor_mul(pm[:], pme[:], bm[:])

            pa = pn.tile([GT, NG * D], F32, tag="pn")
            prs = pn.tile([GT, NG], F32, tag="pn")
            for g in range(NG):
                nc.tensor.matmul(
                    pa[:, g * D:(g + 1) * D],
                    pm[:, g * GTP:g * GTP + GT],
                    vb[:, g * D:(g + 1) * D],
                )
                nc.tensor.matmul(
                    prs[:, g:g + 1], pm[:, g * GTP:g * GTP + GT], onec[:]
                )
            rsv = sp.tile([GT, NG], F32, tag="rsv")
            nc.vector.reciprocal(rsv[:], prs[:])
            scv = sp.tile([GT, NG], F32, tag="scv")
            nc.vector.tensor_mul(scv[:], rsv[:], mk[:])
            y = wp.tile([GT, NG * D], F32, tag="y")
            for g in range(NG):
                nc.vector.scalar_tensor_tensor(
                    y[:, g * D:(g + 1) * D], pa[:, g * D:(g + 1) * D],
                    scv[:, g:g + 1], x[:, g * D:(g + 1) * D],
                    op0=ALU.mult, op1=ALU.add,
                )

            x2 = wp.tile([GT, NG * D], F32, tag="x2")
            layer_norm(
                y,
                g1t[l * GT:(l + 1) * GT, :] if use_g1b1 else None,
                b1t[l * GT:(l + 1) * GT, :] if use_g1b1 else None,
                x2,
            )

            x2T = transpose4(x2, "x2")
            pd = pn.tile([GT, NG * D], F32, tag="pn")
            for g in range(NG):
                nc.tensor.matmul(
                    pd[:, g * D:(g + 1) * D],
                    x2T[:, g * GTP:g * GTP + GT], wd,
                )
            y2 = wp.tile([GT, NG * D], F32, tag="y2")
            nc.vector.tensor_add(y2[:], pd[:], x2[:])
            if use_bd:
                y23 = y2[:].rearrange("p (g d) -> p g d", g=NG)
                nc.vector.tensor_tensor(
                    y23, y23, bcg(bdt[l * GT:(l + 1) * GT, :], D), op=ALU.add
                )
            xn = wp.tile([GT, NG * D], F32, tag="x")
            layer_norm(
                y2,
                g2t[l * GT:(l + 1) * GT, :] if use_g2b2 else None,
                b2t[l * GT:(l + 1) * GT, :] if use_g2b2 else None,
                xn,
            )
            return xn

        def out_proj(it, x):
            xoT = transpose4(x, "xo")
            xoTb = wp.tile([D, TW], BF16, tag="xoTb")
            nc.vector.tensor_copy(xoTb[:], xoT[:])
            po = pn.tile([GT, NG * V], F32, tag="pn")
            for g in range(NG):
                nc.tensor.matmul(
                    po[:, g * V:(g + 1) * V],
                    xoTb[:, g * GTP:g * GTP + GT], wo[:],
                )
            ot = wp.tile([GT, NG * V], F32, tag="ot")
            if use_bo:
                o3 = ot[:].rearrange("p (g v) -> p g v", g=NG)
                nc.vector.tensor_tensor(
                    o3, po[:].rearrange("p (g v) -> p g v", g=NG),
                    bcg(bot, V), op=ALU.add,
                )
            else:
                nc.vector.tensor_copy(ot[:], po[:])
            nc.sync.dma_start(out_d[bass.ts(it, GT), :], ot[:])

        # Two tiles' chains interleaved at layer granularity: the pool-tag
        # rings alternate between the two independent chains so the
        # scheduler can overlap them across engines.
        IL = 4
        assert n_tiles % IL == 0
        with tc.For_i(0, n_tiles, IL, staggered_reset=True) as it:
            states = [dma_embed(it + u) for u in range(IL)]
            xs = [s[0] for s in states]
            mks = [s[1] for s in states]
            for l in range(L):
                for u in range(IL):
                    xs[u] = layer_fn(l, xs[u], mks[u])
            for u in range(IL):
                out_proj(it + u, xs[u])

    nc.compile()
    return nc


def prep_consts(inputs):
    """Host-side constant packing. Returns (consts dict, flags dict)."""
    f = lambda a: np.asarray(a, np.float32)
    We, be = f(inputs["We"]), f(inputs["be"])
    Wp, bp = f(inputs["Wp"]), f(inputs["bp"])
    Wsg, bsg = f(inputs["Wsg"]), f(inputs["bsg"])
    Wq, bq = f(inputs["Wq"]), f(inputs["bq"])
    Wk, bk = f(inputs["Wk"]), f(inputs["bk"])
    Wv, bv = f(inputs["Wv"]), f(inputs["bv"])
    Wd, bd = f(inputs["Wd"]), f(inputs["bd"])
    g1, b1 = f(inputs["g1"]), f(inputs["b1"])
    g2, b2 = f(inputs["g2"]), f(inputs["b2"])
    Wo, bo = f(inputs["Wo"]), f(inputs["bo"])

    wqkd = np.empty((D, L * 3 * D), np.float32)
    wvv = np.empty((D, L * D), np.float32)
    for l in range(L):
        wqkd[:, (l * 3 + 0) * D:(l * 3 + 1) * D] = Wq[l]
        wqkd[:, (l * 3 + 1) * D:(l * 3 + 2) * D] = Wk[l]
        wqkd[:, (l * 3 + 2) * D:(l * 3 + 3) * D] = Wd[l]
        wvv[:, l * D:(l + 1) * D] = Wv[l]
    wcat = np.concatenate([We, Wsg], axis=0)  # [33, 128]
    pos_tot = Wp + (be + bp + bsg)[None, :]   # [17, 128]
    pos4 = np.tile(np.tile(pos_tot, (G, 1)), (1, NG))  # [119, 4*128]
    bqk = np.stack([*bq, *bk], axis=1)  # [128, 2L]
    blockmask = np.kron(np.eye(G, dtype=np.float32), np.ones((S, S), np.float32))
    bm4 = np.tile(blockmask, (1, NG))

    flags = dict(
        use_bv=bool(np.any(bv)), use_bd=bool(np.any(bd)),
        use_g1b1=bool(np.any(g1 != 1) or np.any(b1)),
        use_g2b2=bool(np.any(g2 != 1) or np.any(b2)),
        use_bo=bool(np.any(bo)),
    )
    consts = {
        "wqkd": wqkd,
        "wv": wvv.astype(NPBF),
        "wcat": wcat.astype(np.float32),
        "wo": Wo.astype(NPBF),
        "iden": np.eye(GT, dtype=np.float32),
        "bm": bm4.astype(NPBF),
        "onec": np.ones((GT, 1), np.float32).astype(NPBF),
        "pos4": pos4.astype(np.float32),
        "negc": np.full((GT, 1), -EXPC, np.float32),
        "bqk": bqk.astype(np.float32),
    }
    if flags["use_bv"]:
        consts["bvt"] = np.concatenate([np.tile(bv[l][None, :], (GT, 1)) for l in range(L)])
    if flags["use_bd"]:
        consts["bdt"] = np.concatenate([np.tile(bd[l][None, :], (GT, 1)) for l in range(L)])
    if flags["use_g1b1"]:
        consts["g1t"] = np.concatenate([np.tile(g1[l][None, :], (GT, 1)) for l in range(L)])
        consts["b1t"] = np.concatenate([np.tile(b1[l][None, :], (GT, 1)) for l in range(L)])
    if flags["use_g2b2"]:
        consts["g2t"] = np.concatenate([np.tile(g2[l][None, :], (GT, 1)) for l in range(L)])
        consts["b2t"] = np.concatenate([np.tile(b2[l][None, :], (GT, 1)) for l in range(L)])
    if flags["use_bo"]:
        consts["bot"] = np.tile(bo[None, :], (GT, 1)).astype(np.float32)
    return consts, flags


def shard_activations(X, mask_in, seg_in, n_tiles=NT, ncores=NCORES):
    """Pad batch, fuse X|seg, reorder to [rows=tiles*119, NG*33] per core."""
    Bp = ncores * n_tiles * TS
    f = lambda a: np.asarray(a, np.float32)
    X, mask_in, seg_in = f(X), f(mask_in), f(seg_in)
    npad = Bp - X.shape[0]
    assert npad >= 0
    pad = lambda a: np.concatenate([a, a[:npad]], axis=0) if npad else a
    Xp, mp, sp_ = pad(X), pad(mask_in), pad(seg_in)
    xcat = np.concatenate([Xp, sp_], axis=-1)  # [Bp, S, 33]

    def core_layout(a, w):
        # [spc*S tokens, w] -> [n_tiles, NG, 119, w] -> [n_tiles, 119, NG, w]
        a = a.reshape(n_tiles, NG, GT, w).transpose(0, 2, 1, 3)
        return np.ascontiguousarray(a.reshape(n_tiles * GT, NG * w))

    spc = n_tiles * TS
    xcs, mks = [], []
    for c in range(ncores):
        sl = slice(c * spc, (c + 1) * spc)
        # [spc, S, w] tokens -> [tiles, NG groups, 119 tok, w] -> rows=(tile,tok)
        xcs.append(core_layout(xcat[sl].reshape(spc * S, XW), XW))
        mks.append(core_layout(mp[sl].reshape(spc * S, 1), 1))
    return xcs, mks, npad


def unshard_output(outs, n_tiles=NT, ncores=NCORES, npad=0):
    """outs: list per core of [rows, NG*V] -> [B, S, V]."""
    parts = []
    for o in outs:
        o = o.reshape(n_tiles, GT, NG, V).transpose(0, 2, 1, 3)
        parts.append(o.reshape(n_tiles * TS, S, V))
    full = np.concatenate(parts, axis=0)
    if npad:
        full = full[:-npad]
    return np.ascontiguousarray(full.astype(np.float32))


_GRAPH_CACHE = {}


def get_graph(n_tiles, flags):
    key = (n_tiles, tuple(sorted(flags.items())))
    if key not in _GRAPH_CACHE:
        _GRAPH_CACHE[key] = build_graph(n_tiles, **flags)
    return _GRAPH_CACHE[key]


def kernel(**inputs):
    consts, flags = prep_consts(inputs)
    xcs, mks, npad = shard_activations(inputs["X"], inputs["mask_in"], inputs["seg_in"])
    nc = get_graph(NT, flags)
    in_maps = [{"xc": xcs[c], "mk": mks[c], **consts} for c in range(NCORES)]
    res = bass_utils.run_bass_kernel_spmd(nc, in_maps, core_ids=list(range(NCORES)))
    outs = [res.results[c]["out"] for c in range(NCORES)]
    return unshard_output(outs, npad=npad)


# revision 36
# speedup vs baseline: 8.9926x; 8.9926x over previous
"""Bass/Trainium2 kernel for nn_ArtistBERT (dense transformer, 6 layers, D=128, S=17).

Data-parallel across 8 NeuronCores: batch is padded 16384 -> 16576 samples and
split 2128/core. Per core, samples are processed in 76 tiles of 28 samples
(4 groups of 7 samples = 119 tokens each). The residual stream lives in
"natural" token-major layout [119 tokens, 4*128] so LayerNorm / softmax-scale /
mask are free-axis ops on DVE/ACT; matmuls take transposed operands produced by
PE transposes. All matmul operands are bf16 (PSUM accumulates f32).

Attention per group g (119 tokens, block-diagonal over 7 samples):
  scoresT[k,q] = kb_g^T-contraction  (lhsT=kb_g, rhs=qb_g)
  pm = exp(scoresT - 30) * blockmask           (softmax max-sub replaced by
  attn_unnorm[q,d] = pm^T @ v_nat               constant shift; normalization
  rowsum[q] = pm^T @ ones                       deferred and folded with the
  y = attn_unnorm * mask/rowsum + x (residual)  input keep-mask)
"""

import os
import sys

import numpy as np

try:
    import concourse.bass as bass  # noqa: F401
except ImportError:  # harness runs from a bare directory
    sys.path.insert(0, "/opt/trn_rl_repo")

import ml_dtypes
from contextlib import ExitStack

import concourse.bass as bass
import concourse.tile as tile
from concourse import bacc, mybir
from concourse import bass_utils

# Force every ACT instruction onto the natural_log_exp_and_others table set
# (index 6 in act_info.json): it contains every function this kernel uses
# (exp, ln, square, relu, copy, identity), so the table load hoists out of
# the tile loop instead of thrashing ~2.7us per set switch.
_ONESET = "natural_log_exp_and_others"
_orig_gat = bacc.get_activation_tables


def _gat_oneset(arch):
    t = _orig_gat(arch)
    assert _ONESET in t
    return {k: (v if k == _ONESET else set()) for k, v in t.items()}


bacc.get_activation_tables = _gat_oneset

F32 = mybir.dt.float32
F32R = mybir.dt.float32r
BF16 = mybir.dt.bfloat16
NPBF = ml_dtypes.bfloat16
AF = mybir.ActivationFunctionType
ALU = mybir.AluOpType

B, S, V, D, NSEG, L = 16384, 17, 29, 128, 4, 6
EPS = 1e-5
EXPC = 30.0  # constant shift inside exp(); cancels in the softmax ratio

NCORES = 8
G = 7            # samples per attention group
GT = G * S       # 119 tokens per group
NG = 4           # groups per tile
TS = NG * G      # 28 samples per tile
NT = 76          # tiles per core (divisible by the 4-wide interleave)
SPC = NT * TS    # 2072 samples per core
BPAD = NCORES * SPC  # 16576
XW = V + NSEG    # 33 fused X|seg feature width


def build_graph(n_tiles, use_bv, use_bd, use_g1b1, use_g2b2, use_bo):
    nc = bacc.Bacc(
        "TRN2",
        target_bir_lowering=False,
        debug=False,
        enable_asserts=True,
        num_devices=NCORES,
    )
    rows = n_tiles * GT
    GTP = GT  # group stride in transposed layout
    TW = NG * GTP

    xc_d = nc.dram_tensor("xc", [rows, NG * XW], F32, kind="ExternalInput").ap()
    mk_d = nc.dram_tensor("mk", [rows, NG], F32, kind="ExternalInput").ap()
    wqkd_d = nc.dram_tensor("wqkd", [D, L * 3 * D], F32R, kind="ExternalInput").ap()
    wv_d = nc.dram_tensor("wv", [D, L * D], BF16, kind="ExternalInput").ap()
    wcat_d = nc.dram_tensor("wcat", [XW, D], F32R, kind="ExternalInput").ap()
    wo_d = nc.dram_tensor("wo", [D, V], BF16, kind="ExternalInput").ap()
    iden_d = nc.dram_tensor("iden", [GT, GT], F32, kind="ExternalInput").ap()
    bm_d = nc.dram_tensor("bm", [GT, TW], BF16, kind="ExternalInput").ap()
    onec_d = nc.dram_tensor("onec", [GT, 1], BF16, kind="ExternalInput").ap()
    pos_d = nc.dram_tensor("pos4", [GT, NG * D], F32, kind="ExternalInput").ap()
    negc_d = nc.dram_tensor("negc", [GT, 1], F32, kind="ExternalInput").ap()
    bqk_d = nc.dram_tensor("bqk", [D, 2 * L], F32, kind="ExternalInput").ap()
    bv_d = bd_d = g1_d = b1_d = g2_d = b2_d = bo_d = None
    if use_bv:
        bv_d = nc.dram_tensor("bvt", [L * GT, D], F32, kind="ExternalInput").ap()
    if use_bd:
        bd_d = nc.dram_tensor("bdt", [L * GT, D], F32, kind="ExternalInput").ap()
    if use_g1b1:
        g1_d = nc.dram_tensor("g1t", [L * GT, D], F32, kind="ExternalInput").ap()
        b1_d = nc.dram_tensor("b1t", [L * GT, D], F32, kind="ExternalInput").ap()
    if use_g2b2:
        g2_d = nc.dram_tensor("g2t", [L * GT, D], F32, kind="ExternalInput").ap()
        b2_d = nc.dram_tensor("b2t", [L * GT, D], F32, kind="ExternalInput").ap()
    if use_bo:
        bo_d = nc.dram_tensor("bot", [GT, V], F32, kind="ExternalInput").ap()
    out_d = nc.dram_tensor("out", [rows, NG * V], F32, kind="ExternalOutput").ap()

    with tile.TileContext(nc) as tc, ExitStack() as ctx:
        cp = ctx.enter_context(tc.tile_pool(name="consts", bufs=1))
        wp = ctx.enter_context(tc.tile_pool(name="work", bufs=4))
        sp = ctx.enter_context(tc.tile_pool(name="small", bufs=8))
        pp = ctx.enter_context(tc.tile_pool(name="ps", bufs=4, space="PSUM"))
        pn = ctx.enter_context(tc.tile_pool(name="pn", bufs=4, space="PSUM"))

        def cload(name, ap_d, shape, dt):
            t = cp.tile(shape, dt, tag=name)
            nc.sync.dma_start(t[:], ap_d)
            return t

        wqkd = cload("wqkd", wqkd_d, [D, L * 3 * D], F32R)
        wv = cload("wv", wv_d, [D, L * D], BF16)
        wcat = cload("wcat", wcat_d, [XW, D], F32R)
        wo = cload("wo", wo_d, [D, V], BF16)
        iden = cload("iden", iden_d, [GT, GT], F32)
        bm = cload("bm", bm_d, [GT, TW], BF16)
        onec = cload("onec", onec_d, [GT, 1], BF16)
        pos4 = cload("pos4", pos_d, [GT, NG * D], F32)
        negc = cload("negc", negc_d, [GT, 1], F32)
        bqk = cload("bqk", bqk_d, [D, 2 * L], F32)
        bvt = cload("bvt", bv_d, [L * GT, D], F32) if use_bv else None
        bdt = cload("bdt", bd_d, [L * GT, D], F32) if use_bd else None
        g1t = cload("g1t", g1_d, [L * GT, D], F32) if use_g1b1 else None
        b1t = cload("b1t", b1_d, [L * GT, D], F32) if use_g1b1 else None
        g2t = cload("g2t", g2_d, [L * GT, D], F32) if use_g2b2 else None
        b2t = cload("b2t", b2_d, [L * GT, D], F32) if use_g2b2 else None
        bot = cload("bot", bo_d, [GT, V], F32) if use_bo else None

        def bc3(t2d, w):
            # [GT, NG] -> [GT, NG, w] with 0-stride last dim
            return t2d[:].unsqueeze(2).broadcast_to([GT, NG, w])

        def bcg(t2d, w):
            # [GT, w] const -> [GT, NG, w] broadcast over groups
            return t2d.unsqueeze(1).broadcast_to([GT, NG, w])

        def layer_norm(y, gt, bt, out_f32):
            """y: sbuf f32 [GT, NG*D] -> out_f32. var = E[y^2] - mu^2."""
            y3 = y[:].rearrange("p (g d) -> p g d", g=NG)
            s1 = sp.tile([GT, NG], F32, tag="s1")
            nc.vector.reduce_sum(s1[:], y3, axis=mybir.AxisListType.X)
            mu = sp.tile([GT, NG], F32, tag="mu")
            nc.vector.tensor_scalar(mu[:], s1[:], 1.0 / D, None, op0=ALU.mult)
            sq = wp.tile([GT, NG * D], F32, tag="sq")
            nc.scalar.activation(sq[:], y[:], AF.Square)
            s2 = sp.tile([GT, NG], F32, tag="s2")
            nc.vector.reduce_sum(
                s2[:], sq[:].rearrange("p (g d) -> p g d", g=NG),
                axis=mybir.AxisListType.X,
            )
            mu2 = sp.tile([GT, NG], F32, tag="mu2")
            nc.vector.tensor_mul(mu2[:], mu[:], mu[:])
            vv = sp.tile([GT, NG], F32, tag="vv")
            nc.vector.tensor_scalar(
                vv[:], s2[:], 1.0 / D, EPS, op0=ALU.mult, op1=ALU.add
            )
            nc.vector.tensor_sub(vv[:], vv[:], mu2[:])
            # rstd = exp(-0.5*ln(var)): keeps every ACT func in the
            # natural_log_exp table set (no table-switch stalls)
            lnv = sp.tile([GT, NG], F32, tag="lnv")
            nc.scalar.activation(lnv[:], vv[:], AF.Ln)
            rst = sp.tile([GT, NG], F32, tag="rst")
            nc.scalar.activation(rst[:], lnv[:], AF.Exp, scale=-0.5)
            for g in range(NG):
                nc.vector.tensor_scalar(
                    out_f32[:, g * D:(g + 1) * D], y[:, g * D:(g + 1) * D],
                    mu[:, g:g + 1], rst[:, g:g + 1],
                    op0=ALU.subtract, op1=ALU.mult,
                )
            o3 = out_f32[:].rearrange("p (g d) -> p g d", g=NG)
            if gt is not None:
                nc.vector.tensor_tensor(o3, o3, bcg(gt, D), op=ALU.mult)
            if bt is not None:
                nc.vector.tensor_tensor(o3, o3, bcg(bt, D), op=ALU.add)

        def transpose4(src_f32, tag):
            """src [GT, NG*D] f32 -> f32r [D, TW]; group g at cols g*GTP,
            col 119 of each group is a defined zero (iden pad column)."""
            pt = pp.tile([D, TW], F32, tag="ps")
            for g in range(NG):
                nc.tensor.transpose(
                    pt[:, g * GTP:(g + 1) * GTP], src_f32[:, g * D:(g + 1) * D],
                    iden[:],
                )
            xbT = wp.tile([D, TW], F32R, tag=tag + "T")
            nc.scalar.copy(xbT[:], pt[:])
            return xbT

        def dma_embed(it):
            xc = wp.tile([GT, NG * XW], F32, tag="xc")
            nc.sync.dma_start(xc[:], xc_d[bass.ts(it, GT), :])
            mk = wp.tile([GT, NG], F32, tag="mk")
            nc.sync.dma_start(mk[:], mk_d[bass.ts(it, GT), :])

            pxt = pp.tile([XW, TW], F32, tag="ps")
            for g in range(NG):
                nc.tensor.transpose(
                    pxt[:, g * GTP:(g + 1) * GTP], xc[:, g * XW:(g + 1) * XW],
                    iden[:],
                )
            xcT = wp.tile([XW, TW], F32R, tag="xcT")
            nc.scalar.copy(xcT[:], pxt[:])
            px = pn.tile([GT, NG * D], F32, tag="pn")
            for g in range(NG):
                nc.tensor.matmul(
                    px[:, g * D:(g + 1) * D],
                    xcT[:, g * GTP:g * GTP + GT],
                    wcat[:],
                )
            x = wp.tile([GT, NG * D], F32, tag="x")
            nc.vector.tensor_add(x[:], px[:], pos4[:])
            return x, mk

        def layer_fn(l, x, mk):
            wq = wqkd[:, (l * 3 + 0) * D:(l * 3 + 1) * D]
            wk = wqkd[:, (l * 3 + 1) * D:(l * 3 + 2) * D]
            wd = wqkd[:, (l * 3 + 2) * D:(l * 3 + 3) * D]
            wvl = wv[:, l * D:(l + 1) * D]

            xbT = transpose4(x, "x1")

            pq = pp.tile([D, TW], F32, tag="ps")
            nc.tensor.matmul(pq[:], wq, xbT[:])
            qb = wp.tile([D, TW], F32, tag="qb")
            nc.scalar.activation(qb[:], pq[:], AF.Relu, bias=bqk[:, l:l + 1])
            pk = pp.tile([D, TW], F32, tag="ps")
            nc.tensor.matmul(pk[:], wk, xbT[:])
            kb = wp.tile([D, TW], F32, tag="kb")
            nc.scalar.activation(kb[:], pk[:], AF.Relu, bias=bqk[:, L + l:L + l + 1])

            xbTb = wp.tile([D, TW], BF16, tag="xbTb")
            nc.vector.tensor_copy(xbTb[:], xbT[:])
            pv = pn.tile([GT, NG * D], F32, tag="pn")
            for g in range(NG):
                nc.tensor.matmul(
                    pv[:, g * D:(g + 1) * D],
                    xbTb[:, g * GTP:g * GTP + GT], wvl,
                )
            vb = wp.tile([GT, NG * D], BF16, tag="vb")
            if use_bv:
                vf = wp.tile([GT, NG * D], F32, tag="vf")
                v3 = vf[:].rearrange("p (g d) -> p g d", g=NG)
                nc.vector.tensor_tensor(
                    v3, pv[:].rearrange("p (g d) -> p g d", g=NG),
                    bcg(bvt[l * GT:(l + 1) * GT, :], D), op=ALU.add,
                )
                nc.scalar.activation(vb[:], vf[:], AF.Relu)
            else:
                nc.scalar.activation(vb[:], pv[:], AF.Relu)

            pss = pn.tile([GT, TW], F32, tag="pn")
            for g in range(NG):
                nc.tensor.matmul(
                    pss[:, g * GTP:(g + 1) * GTP],
                    kb[:, g * GTP:g * GTP + GT],
                    qb[:, g * GTP:(g + 1) * GTP],
                )
            pme = wp.tile([GT, TW], BF16, tag="pme")
            nc.scalar.activation(pme[:], pss[:], AF.Exp, bias=negc[:])
            pm = wp.tile([GT, TW], BF16, tag="pm")
            nc.vector.tensor_mul(pm[:], pme[:], bm[:])

            pa = pn.tile([GT, NG * D], F32, tag="pn")
            prs = pn.tile([GT, NG], F32, tag="pn")
            for g in range(NG):
                nc.tensor.matmul(
                    pa[:, g * D:(g + 1) * D],
                    pm[:, g * GTP:g * GTP + GT],
                    vb[:, g * D:(g + 1) * D],
                )
                nc.tensor.matmul(
                    prs[:, g:g + 1], pm[:, g * GTP:g * GTP + GT], onec[:]
                )
            rsv = sp.tile([GT, NG], F32, tag="rsv")
            nc.vector.reciprocal(rsv[:], prs[:])
            scv = sp.tile([GT, NG], F32, tag="scv")
            nc.vector.tensor_mul(scv[:], rsv[:], mk[:])
            y = wp.tile([GT, NG * D], F32, tag="y")
            for g in range(NG):
                nc.vector.scalar_tensor_tensor(
                    y[:, g * D:(g + 1) * D], pa[:, g * D:(g + 1) * D],
                    scv[:, g:g + 1], x[:, g * D:(g + 1) * D],
                    op0=ALU.mult, op1=ALU.add,
                )

            x2 = wp.tile([GT, NG * D], F32, tag="x2")
            layer_norm(
                y,
                g1t[l * GT:(l + 1) * GT, :] if use_g1b1 else None,
                b1t[l * GT:(l + 1) * GT, :] if use_g1b1 else None,
                x2,
            )

            x2T = transpose4(x2, "x2")
            pd = pn.tile([GT, NG * D], F32, tag="pn")
            for g in range(NG):
                nc.tensor.matmul(
                    pd[:, g * D:(g + 1) * D],
                    x2T[:, g * GTP:g * GTP + GT], wd,
                )
            y2 = wp.tile([GT, NG * D], F32, tag="y2")
            nc.vector.tensor_add(y2[:], pd[:], x2[:])
            if use_bd:
                y23 = y2[:].rearrange("p (g d) -> p g d", g=NG)
                nc.vector.tensor_tensor(
                    y23, y23, bcg(bdt[l * GT:(l + 1) * GT, :], D), op=ALU.add
                )
            xn = wp.tile([GT, NG * D], F32, tag="x")
            layer_norm(
                y2,
                g2t[l * GT:(l + 1) * GT, :] if use_g2b2 else None,
                b2t[l * GT:(l + 1) * GT, :] if use_g2b2 else None,
                xn,
            )
            return xn

        def out_proj(it, x):
            xoT = transpose4(x, "xo")
            xoTb = wp.tile([D, TW], BF16, tag="xoTb")
            nc.vector.tensor_copy(xoTb[:], xoT[:])
            po = pn.tile([GT, NG * V], F32, tag="pn")
            for g in range(NG):
                nc.tensor.matmul(
                    po[:, g * V:(g + 1) * V],
                    xoTb[:, g * GTP:g * GTP + GT], wo[:],
                )
            ot = wp.tile([GT, NG * V], F32, tag="ot")
            if use_bo:
                o3 = ot[:].rearrange("p (g v) -> p g v", g=NG)
                nc.vector.tensor_tensor(
                    o3, po[:].rearrange("p (g v) -> p g v", g=NG),
                    bcg(bot, V), op=ALU.add,
                )
            else:
                nc.vector.tensor_copy(ot[:], po[:])
            nc.sync.dma_start(out_d[bass.ts(it, GT), :], ot[:])

        # Two tiles' chains interleaved at layer granularity: the pool-tag
        # rings alternate between the two independent chains so the
        # scheduler can overlap them across engines.
        IL = 4
        assert n_tiles % IL == 0
        with tc.For_i(0, n_tiles, IL, staggered_reset=True) as it:
            states = [dma_embed(it + u) for u in range(IL)]
            xs = [s[0] for s in states]
            mks = [s[1] for s in states]
            for l in range(L):
                for u in range(IL):
                    xs[u] = layer_fn(l, xs[u], mks[u])
            for u in range(IL):
                out_proj(it + u, xs[u])

    nc.compile()
    return nc


def prep_consts(inputs):
    """Host-side constant packing. Returns (consts dict, flags dict)."""
    f = lambda a: np.asarray(a, np.float32)
    We, be = f(inputs["We"]), f(inputs["be"])
    Wp, bp = f(inputs["Wp"]), f(inputs["bp"])
    Wsg, bsg = f(inputs["Wsg"]), f(inputs["bsg"])
    Wq, bq = f(inputs["Wq"]), f(inputs["bq"])
    Wk, bk = f(inputs["Wk"]), f(inputs["bk"])
    Wv, bv = f(inputs["Wv"]), f(inputs["bv"])
    Wd, bd = f(inputs["Wd"]), f(inputs["bd"])
    g1, b1 = f(inputs["g1"]), f(inputs["b1"])
    g2, b2 = f(inputs["g2"]), f(inputs["b2"])
    Wo, bo = f(inputs["Wo"]), f(inputs["bo"])

    wqkd = np.empty((D, L * 3 * D), np.float32)
    wvv = np.empty((D, L * D), np.float32)
    for l in range(L):
        wqkd[:, (l * 3 + 0) * D:(l * 3 + 1) * D] = Wq[l]
        wqkd[:, (l * 3 + 1) * D:(l * 3 + 2) * D] = Wk[l]
        wqkd[:, (l * 3 + 2) * D:(l * 3 + 3) * D] = Wd[l]
        wvv[:, l * D:(l + 1) * D] = Wv[l]
    wcat = np.concatenate([We, Wsg], axis=0)  # [33, 128]
    pos_tot = Wp + (be + bp + bsg)[None, :]   # [17, 128]
    pos4 = np.tile(np.tile(pos_tot, (G, 1)), (1, NG))  # [119, 4*128]
    bqk = np.stack([*bq, *bk], axis=1)  # [128, 2L]
    blockmask = np.kron(np.eye(G, dtype=np.float32), np.ones((S, S), np.float32))
    bm4 = np.tile(blockmask, (1, NG))

    flags = dict(
        use_bv=bool(np.any(bv)), use_bd=bool(np.any(bd)),
        use_g1b1=bool(np.any(g1 != 1) or np.any(b1)),
        use_g2b2=bool(np.any(g2 != 1) or np.any(b2)),
        use_bo=bool(np.any(bo)),
    )
    consts = {
        "wqkd": wqkd,
        "wv": wvv.astype(NPBF),
        "wcat": wcat.astype(np.float32),
        "wo": Wo.astype(NPBF),
        "iden": np.eye(GT, dtype=np.float32),
        "bm": bm4.astype(NPBF),
        "onec": np.ones((GT, 1), np.float32).astype(NPBF),
        "pos4": pos4.astype(np.float32),
        "negc": np.full((GT, 1), -EXPC, np.float32),
        "bqk": bqk.astype(np.float32),
    }
    if flags["use_bv"]:
        consts["bvt"] = np.concatenate([np.tile(bv[l][None, :], (GT, 1)) for l in range(L)])
    if flags["use_bd"]:
        consts["bdt"] = np.concatenate([np.tile(bd[l][None, :], (GT, 1)) for l in range(L)])
    if flags["use_g1b1"]:
        consts["g1t"] = np.concatenate([np.tile(g1[l][None, :], (GT, 1)) for l in range(L)])
        consts["b1t"] = np.concatenate([np.tile(b1[l][None, :], (GT, 1)) for l in range(L)])
    if flags["use_g2b2"]:
        consts["g2t"] = np.concatenate([np.tile(g2[l][None, :], (GT, 1)) for l in range(L)])
        consts["b2t"] = np.concatenate([np.tile(b2[l][None, :], (GT, 1)) for l in range(L)])
    if flags["use_bo"]:
        consts["bot"] = np.tile(bo[None, :], (GT, 1)).astype(np.float32)
    return consts, flags


def shard_activations(X, mask_in, seg_in, n_tiles=NT, ncores=NCORES):
    """Pad batch, fuse X|seg, reorder to [rows=tiles*119, NG*33] per core."""
    Bp = ncores * n_tiles * TS
    f = lambda a: np.asarray(a, np.float32)
    X, mask_in, seg_in = f(X), f(mask_in), f(seg_in)
    npad = Bp - X.shape[0]
    assert npad >= 0
    pad = lambda a: np.concatenate([a, a[:npad]], axis=0) if npad else a
    Xp, mp, sp_ = pad(X), pad(mask_in), pad(seg_in)
    xcat = np.concatenate([Xp, sp_], axis=-1)  # [Bp, S, 33]

    def core_layout(a, w):
        # [spc*S tokens, w] -> [n_tiles, NG, 119, w] -> [n_tiles, 119, NG, w]
        a = a.reshape(n_tiles, NG, GT, w).transpose(0, 2, 1, 3)
        return np.ascontiguousarray(a.reshape(n_tiles * GT, NG * w))

    spc = n_tiles * TS
    xcs, mks = [], []
    for c in range(ncores):
        sl = slice(c * spc, (c + 1) * spc)
        # [spc, S, w] tokens -> [tiles, NG groups, 119 tok, w] -> rows=(tile,tok)
        xcs.append(core_layout(xcat[sl].reshape(spc * S, XW), XW))
        mks.append(core_layout(mp[sl].reshape(spc * S, 1), 1))
    return xcs, mks, npad


def unshard_output(outs, n_tiles=NT, ncores=NCORES, npad=0):
    """outs: list per core of [rows, NG*V] -> [B, S, V]."""
    parts = []
    for o in outs:
        o = o.reshape(n_tiles, GT, NG, V).transpose(0, 2, 1, 3)
        parts.append(o.reshape(n_tiles * TS, S, V))
    full = np.concatenate(parts, axis=0)
    if npad:
        full = full[:-npad]
    return np.ascontiguousarray(full.astype(np.float32))


_GRAPH_CACHE = {}


def get_graph(n_tiles, flags):
    key = (n_tiles, tuple(sorted(flags.items())))
    if key not in _GRAPH_CACHE:
        _GRAPH_CACHE[key] = build_graph(n_tiles, **flags)
    return _GRAPH_CACHE[key]


def kernel(**inputs):
    consts, flags = prep_consts(inputs)
    xcs, mks, npad = shard_activations(inputs["X"], inputs["mask_in"], inputs["seg_in"])
    nc = get_graph(NT, flags)
    in_maps = [{"xc": xcs[c], "mk": mks[c], **consts} for c in range(NCORES)]
    res = bass_utils.run_bass_kernel_spmd(nc, in_maps, core_ids=list(range(NCORES)))
    outs = [res.results[c]["out"] for c in range(NCORES)]
    return unshard_output(outs, npad=npad)
